# revision 1
# baseline (speedup 1.0000x reference)
"""Trainium2 Bass kernel for the FNO-style FourierLayer.

  x: [8, 512, 512, 32] f32 -> rfft2 over (h, w) -> keep 32x32 modes ->
  per-mode (C x C) channel mix with W[32, 32, 32, 32] -> zero-pad -> irfft2.

Strategy: data-parallel over batch, one sample per NeuronCore (8 cores).
Only 32 of 512 frequencies survive, so instead of an FFT each core runs a
chain of small dense real matmuls against DFT basis matrices (bf16 operands,
fp32 PSUM accumulation):

  A:   P = F^T X         contract h       -> [kxri 64, (w c)]
  T1:  PE transposes     w onto partitions
  B:   raw = G^T PT      contract w       -> [ryky 64, (c rx kx)] psum accum
  Tc:  PE transposes     c onto partitions -> rawT [32, (kx rx ry ky)]
  CMB: complex re/im combine fused into a block-diagonal lhsT build:
       diag[32*kyl + c, kx*64 + kyg*8 + 2*kyl + ri] = low[kx, ky, ri, c]
  C:   per-(kx, ky-group-of-4) matmuls  OL = diag^T W  (256 matmuls of
       32 cols each; out rows (2*kyl+ri) at PSUM quadrant kyg%4)
  OLT: [64 (2ky+ri), (kx d)] assembled by 8 copies from PSUM quadrants
  D:   u = Dab^T OLT      contract 2ky+ri -> u_wc [w 128, (ab kx d)]
  T2:  PE transposes     (ab kx) onto partitions -> uT [64, (w d)]
  E:   out = Einv^T uT    contract (ab kx) -> [h 128, (w d)] -> DMA out

D/T2/E are interleaved per w-chunk so output DMA starts early; PSUM
evacuation is rotated across Vector/Scalar/GpSimd so the PE never stalls
(stalls drop the PE p-state from 2.4 GHz to 1.2 GHz). Input x streams in
half-tile DMA slices so stage A starts ~3.5us in; the PE warms up on a
memset tile to open the p-state ramp without waiting for any DMA.

DFT matrices are built on host from np.fft basis responses (this captures
the irfft Im(DC)-drop convention exactly). x, W and the matrices are cast
to bf16 on host and the output is returned as bf16 and upcast on host,
which halves DMA traffic in both directions.
"""
import numpy as np
import ml_dtypes

import concourse.bass as bass
import concourse.bacc as bacc
import concourse.mybir as mybir
from concourse import tile
from concourse.bass_utils import run_bass_kernel_spmd

B, H, W_, C = 8, 512, 512, 32
MODES = 32
N = 512
NCORES = 8

BF = mybir.dt.bfloat16
F32 = mybir.dt.float32


def _make_consts():
    h = np.arange(N)
    k = np.arange(MODES)
    ang = 2 * np.pi * np.outer(h, k) / N
    F = np.concatenate([np.cos(ang), -np.sin(ang)], axis=1)      # [512, 64]

    eye = np.eye(MODES)
    zc = np.concatenate([eye, np.zeros((MODES, N // 2 + 1 - MODES))], axis=1)
    row_re = np.fft.irfft(zc, n=N, axis=1)                        # [32, 512]
    row_im = np.fft.irfft(1j * zc, n=N, axis=1)

    # rows in interleaved (2*ky + ri) order, matching OLT rows
    Da = np.zeros((64, N))
    Db = np.zeros((64, N))
    Da[0::2] = row_re
    Da[1::2] = row_im
    Db[0::2] = row_im
    Db[1::2] = -row_re

    Einv = np.concatenate([np.cos(ang).T, np.sin(ang).T], axis=0) / N  # [64, 512]

    # F_sb[p, k*64+j] = F[k*128+p, j]
    F_sb = F.reshape(4, 128, 64).transpose(1, 0, 2).reshape(128, 256)
    Dab_sb = np.concatenate([Da, Db], axis=1)                          # [64, 1024]
    ident = np.eye(128)

    # permutation lhsT assembling OLT rows from C-stage psum quadrants:
    # psum tile t holds ky-group kyg = 3t + q at partition rows 32q + r;
    # OLT row = 8*kyg + r. Pt[32q + r, 8*(3t+q) + r] = 1.
    perm = np.zeros((3, 128, 64))
    for kyg in range(8):
        t, q = divmod(kyg, 3)
        for r in range(8):
            perm[t, 32 * q + r, 8 * kyg + r] = 1.0
    return (F_sb.astype(ml_dtypes.bfloat16), Dab_sb.astype(ml_dtypes.bfloat16),
            Einv.astype(ml_dtypes.bfloat16), ident.astype(ml_dtypes.bfloat16),
            perm.astype(ml_dtypes.bfloat16))


def _build_nc():
    F_np, Dab_np, Einv_np, idb_np, perm_np = _make_consts()

    nc = bacc.Bacc()
    x_d = nc.dram_tensor("x", [H, W_ * C], BF, kind="ExternalInput")
    # wpe[kyl*32 + c, kx*256 + kyg*32 + d] = W[kx, kyg*4 + kyl, c, d]
    wpe_d = nc.dram_tensor("wpe", [128, 8192], BF, kind="ExternalInput")
    out_d = nc.dram_tensor("out", [H, W_ * C], BF, kind="ExternalOutput")

    f_c = nc.inline_tensor(F_np, name="f_const")
    dab_c = nc.inline_tensor(Dab_np, name="dab_const")
    einv_c = nc.inline_tensor(Einv_np, name="einv_const")
    idb_c = nc.inline_tensor(idb_np, name="idb_const")
    perm_c = nc.inline_tensor(
        np.ascontiguousarray(perm_np.transpose(1, 0, 2).reshape(128, 192)),
        name="perm_const")

    with tile.TileContext(nc) as tc:
        with (
            tc.tile_pool(name="const", bufs=1) as cpool,
            tc.tile_pool(name="xp", bufs=8) as xpool,
            tc.tile_pool(name="mid", bufs=2) as midpool,
            tc.tile_pool(name="ptp", bufs=2) as ptpool,
            tc.tile_pool(name="wp", bufs=1) as wpool,
            tc.tile_pool(name="sml", bufs=1) as smlpool,
            tc.tile_pool(name="up", bufs=2) as upool,
            tc.tile_pool(name="osb", bufs=4) as opool,
        ):
            # ---- constants ----
            F_sb = cpool.tile([128, 256], BF)
            Dab_sb = cpool.tile([64, 1024], BF)
            Einv_sb = cpool.tile([64, 512], BF)
            ident_bf = cpool.tile([128, 128], BF)
            warm_sb = cpool.tile([128, 128], BF)
            perm_sb = cpool.tile([128, 192], BF)
            wpe_sb = wpool.tile([128, 8192], BF)

            raw_sb = smlpool.tile([64, 2048], BF)
            rawT = smlpool.tile([32, 4096], BF)
            diag = smlpool.tile([128, 2048], BF)
            OL2 = smlpool.tile([128, 3072], BF)
            OLT = smlpool.tile([64, 1024], BF)
            uT = smlpool.tile([64, 16384], BF, tag="bigshare", name="uT")

            # warm tile via memset: no DMA dependency, PE can start ~t=0
            nc.gpsimd.memset(warm_sb[:], 0.25)
            nc.gpsimd.memset(diag[:], 0.0)

            # F first (needed by first A matmul), then x streams in.
            nc.sync.dma_start(F_sb[:], f_c[:])

            # PE warmup: open the p-state ramp while first x tiles fly
            with tc.tile_pool(name="ps_w", bufs=1,
                              space=bass.MemorySpace.PSUM) as psw:
                wps = psw.tile([128, 512], F32, tag="wps", name="wps")
                for wi in range(56):
                    nc.tensor.matmul(
                        wps[:, 0:128], warm_sb[:], warm_sb[:],
                        start=True, stop=True)

            # rotating PSUM->SBUF evacuation (GPSIMD cannot access PSUM,
            # so only DVE + ACT share this work)
            _rot = [nc.vector.tensor_copy, nc.scalar.copy]
            _rix = [0]

            def rcopy(dst, src):
                fn = _rot[_rix[0] % 2]
                _rix[0] += 1
                fn(dst, src)

            with (
                tc.tile_pool(name="ps_acc", bufs=4,
                             space=bass.MemorySpace.PSUM) as psa,
                tc.tile_pool(name="ps_pa", bufs=2,
                             space=bass.MemorySpace.PSUM) as ppa,
                tc.tile_pool(name="ps_pt1", bufs=2,
                             space=bass.MemorySpace.PSUM) as ppt1,
            ):
                # persistent stage-B accumulators
                pb = [psa.tile([64, 512], F32, tag="pb", bufs=4, name=f"pb{i}")
                      for i in range(4)]

                # ============= A + T1 + B, software-pipelined ============
                def emit_A(wq):
                    P_wq = midpool.tile([64, 4096], BF, tag="mid",
                                        name=f"P{wq}")
                    xk = []
                    for k in range(4):
                        t = xpool.tile([128, 4096], BF, tag="xk",
                                       name=f"x{wq}{k}")
                        xk.append(t)
                    # half-tile DMA slices in consumption order: the first
                    # pa group only waits for the first 4 half-slices.
                    for half in range(2):
                        for k in range(4):
                            nc.sync.dma_start(
                                xk[k][:, half * 2048:(half + 1) * 2048],
                                x_d[k * 128:(k + 1) * 128,
                                    wq * 4096 + half * 2048:
                                    wq * 4096 + (half + 1) * 2048])
                    if wq == 0:
                        nc.sync.dma_start(ident_bf[:], idb_c[:])
                    if wq == 1:
                        nc.sync.dma_start(Dab_sb[:], dab_c[:])
                        nc.sync.dma_start(Einv_sb[:], einv_c[:])
                        nc.sync.dma_start(perm_sb[:], perm_c[:])
                    for ns in range(8):
                        pa = ppa.tile([64, 512], F32, tag="pa",
                                      name=f"pa{wq}{ns}")
                        for k in range(4):
                            nc.tensor.matmul(
                                pa[:], F_sb[:, k * 64:(k + 1) * 64],
                                xk[k][:, ns * 512:(ns + 1) * 512],
                                start=(k == 0), stop=(k == 3))
                        rcopy(P_wq[:, ns * 512:(ns + 1) * 512], pa[:])
                    return P_wq

                def emit_T1B(wq, P_wq):
                    PT_wq = ptpool.tile([128, 2048], BF, tag="pt",
                                        name=f"PT{wq}")
                    Pv = P_wq.rearrange("p (w c) -> p w c", c=32)
                    for cg in range(4):
                        pt1 = ppt1.tile([128, 512], BF, tag="pt1",
                                        name=f"pt1_{wq}{cg}")
                        for cl in range(8):
                            c = cg * 8 + cl
                            nc.tensor.transpose(
                                pt1[:, cl * 64:(cl + 1) * 64],
                                Pv[:, :, c], ident_bf[0:64, 0:64])
                        rcopy(PT_wq[:, cg * 512:(cg + 1) * 512], pt1[:])
                    for ns in range(4):
                        nc.tensor.matmul(
                            pb[ns][:], F_sb[:, wq * 64:(wq + 1) * 64],
                            PT_wq[:, ns * 512:(ns + 1) * 512],
                            start=(wq == 0), stop=(wq == 3))

                P_prev = emit_A(0)
                for wq in range(1, 4):
                    P_cur = emit_A(wq)
                    emit_T1B(wq - 1, P_prev)
                    P_prev = P_cur
                # W arrives after all x: off the phase-in critical path,
                # well before stage C needs it.
                nc.sync.dma_start(wpe_sb[:], wpe_d[:])
                emit_T1B(3, P_prev)

                for ns in range(4):
                    rcopy(raw_sb[:, ns * 512:(ns + 1) * 512], pb[ns][:])

            # ====== Tc: c onto partitions; fused combine -> diag; C =======
            # raw_sb[ry*32+ky, c*64 + rx*32 + kx]
            # rawT[c, kx*128 + rx*64 + ry*32 + ky]
            rawv = raw_sb.rearrange("p (c k) -> p c k", k=64)
            rTv = rawT.rearrange("p (kx rx ry kyg kyl) -> p kx rx ry kyg kyl",
                                 kx=32, rx=2, ry=2, kyg=8)
            diag_v = diag.rearrange("p (kx kyg r) -> p kx kyg r",
                                    kx=32, kyg=8)
            with tc.tile_pool(name="ps_c", bufs=2,
                              space=bass.MemorySpace.PSUM) as pcp:
                # C: 256 matmuls, out rows (2*kyl+ri); PSUM out partition
                # base must be 0/32/64, so 3 ky-groups per psum tile.
                # Unused psum rows are memset to 0 (the permutation matmul
                # below multiplies them by 0, and 0*NaN would poison it).
                pC = [pcp.tile([128, 1024], F32, tag="pC", bufs=3,
                               name=f"pC{i}") for i in range(3)]
                for t in range(3):
                    nc.vector.memset(pC[t][:], 0.0)

                with tc.tile_pool(name="ps_tc", bufs=2,
                                  space=bass.MemorySpace.PSUM) as ptcp:
                    for kxg in range(8):
                        ptc = ptcp.tile([32, 512], BF, tag="tc",
                                        name=f"ptc{kxg}")
                        for kxl in range(4):
                            kx = kxg * 4 + kxl
                            for rix in range(2):
                                nc.tensor.transpose(
                                    ptc[:, kxl * 128 + rix * 64:
                                        kxl * 128 + rix * 64 + 64],
                                    rawv[:, :, rix * 32 + kx],
                                    ident_bf[0:64, 0:64])
                        rcopy(rawT[:, kxg * 512:(kxg + 1) * 512], ptc[:])

                def emit_cmb(half):
                    kxs = slice(half * 16, (half + 1) * 16)
                    for kyl in range(4):
                        prow = slice(32 * kyl, 32 * kyl + 32)
                        nc.gpsimd.tensor_tensor(
                            diag_v[prow, kxs, :, 2 * kyl],
                            rTv[:, kxs, 0, 0, :, kyl],
                            rTv[:, kxs, 1, 1, :, kyl],
                            mybir.AluOpType.subtract)
                        nc.gpsimd.tensor_tensor(
                            diag_v[prow, kxs, :, 2 * kyl + 1],
                            rTv[:, kxs, 0, 1, :, kyl],
                            rTv[:, kxs, 1, 0, :, kyl],
                            mybir.AluOpType.add)

                emit_cmb(0)
                emit_cmb(1)

                for kx in range(32):
                    for kyg in range(8):
                        t, q = divmod(kyg, 3)
                        nc.tensor.matmul(
                            pC[t][32 * q:32 * q + 8,
                                  kx * 32:(kx + 1) * 32],
                            diag[:, kx * 64 + kyg * 8:kx * 64 + kyg * 8 + 8],
                            wpe_sb[:, kx * 256 + kyg * 32:
                                   kx * 256 + kyg * 32 + 32],
                            start=True, stop=True)
                for t in range(3):
                    rcopy(OL2[:, t * 1024:(t + 1) * 1024], pC[t][:])

            # assemble OLT rows (8*kyg + r) from the psum quadrant layout
            # with 3 accumulating permutation matmuls (disjoint out rows)
            with tc.tile_pool(name="ps_pm", bufs=1,
                              space=bass.MemorySpace.PSUM) as ppm:
                pm = ppm.tile([64, 1024], F32, tag="pm", name="pm")
                for h in range(2):
                    for t in range(3):
                        nc.tensor.matmul(
                            pm[:, h * 512:(h + 1) * 512],
                            perm_sb[:, t * 64:(t + 1) * 64],
                            OL2[:, t * 1024 + h * 512:
                                t * 1024 + (h + 1) * 512],
                            start=(t == 0), stop=(t == 2))
                rcopy(OLT[:], pm[:])

            # ========== D + T2 + E interleaved per w-chunk ================
            uTv = uT.rearrange("p (w d) -> p w d", d=32)
            with (
                tc.tile_pool(name="ps_d", bufs=2,
                             space=bass.MemorySpace.PSUM) as pdp,
                tc.tile_pool(name="ps_t2", bufs=2,
                             space=bass.MemorySpace.PSUM) as pt2p,
                tc.tile_pool(name="ps_e", bufs=4,
                             space=bass.MemorySpace.PSUM) as pse,
            ):
                def emit_D(wc):
                    u_wc = upool.tile([128, 2048], BF, tag="u",
                                      name=f"u{wc}")
                    for ab in range(2):
                        for ns in range(2):
                            pd = pdp.tile([128, 512], F32, tag="pd",
                                          name=f"pd{wc}{ab}{ns}")
                            nc.tensor.matmul(
                                pd[:],
                                Dab_sb[:, ab * 512 + wc * 128:
                                       ab * 512 + (wc + 1) * 128],
                                OLT[:, ns * 512:(ns + 1) * 512],
                                start=True, stop=True)
                            rcopy(u_wc[:, ab * 1024 + ns * 512:
                                       ab * 1024 + (ns + 1) * 512], pd[:])
                    return u_wc

                def emit_T2(wc, u_wc):
                    uv = u_wc.rearrange("p (ab kx d) -> p ab kx d",
                                        ab=2, d=32)
                    for dg in range(8):
                        pt2 = pt2p.tile([64, 512], BF, tag="pt2",
                                        name=f"pt2_{wc}{dg}")
                        for dl in range(4):
                            d = dg * 4 + dl
                            nc.tensor.transpose(
                                pt2[:, dl * 128:(dl + 1) * 128],
                                uv[:, :, :, d], ident_bf[:])
                        # dest scan (w, dl) <- src cols dl*128 + w
                        p2v = pt2.rearrange("p (dl w) -> p w dl", w=128)
                        rcopy(uTv[:, wc * 128:(wc + 1) * 128,
                                  dg * 4:dg * 4 + 4], p2v[:])

                def emit_E(qb):
                    for hc in range(4):
                        ob = opool.tile([128, 4096], BF, tag="osb",
                                        name=f"ob{hc}{qb}")
                        for sb in range(8):
                            nb = qb * 8 + sb
                            pe_t = pse.tile([128, 512], F32, tag="pse",
                                            name=f"pe{hc}{qb}{sb}")
                            nc.tensor.matmul(
                                pe_t[:],
                                Einv_sb[:, hc * 128:(hc + 1) * 128],
                                uT[:, nb * 512:(nb + 1) * 512],
                                start=True, stop=True)
                            rcopy(ob[:, sb * 512:(sb + 1) * 512], pe_t[:])
                        nc.sync.dma_start(
                            out_d[hc * 128:(hc + 1) * 128,
                                  qb * 4096:(qb + 1) * 4096],
                            ob[:])

                for wc in range(4):
                    u_wc = emit_D(wc)
                    emit_T2(wc, u_wc)
                    emit_E(wc)
    nc.compile()
    return nc


_NC_CACHE = {}


def _get_nc():
    if "nc" not in _NC_CACHE:
        _NC_CACHE["nc"] = _build_nc()
    return _NC_CACHE["nc"]


def _wpe_from_W(W):
    # wpe[kyl*32 + c, kx*256 + kyg*32 + d] = W[kx, kyg*4 + kyl, c, d]
    Wt = np.asarray(W, dtype=np.float32).reshape(32, 8, 4, 32, 32)
    wpe = Wt.transpose(2, 3, 0, 1, 4).reshape(128, 8192)
    return np.ascontiguousarray(wpe.astype(ml_dtypes.bfloat16))


def kernel(x, W):
    xb = np.asarray(x).reshape(NCORES, H, W_ * C).astype(ml_dtypes.bfloat16)
    wpe = _wpe_from_W(W)
    nc = _get_nc()
    in_maps = [{"x": np.ascontiguousarray(xb[i]), "wpe": wpe}
               for i in range(NCORES)]
    res = run_bass_kernel_spmd(nc, in_maps, list(range(NCORES))).results
    out = np.stack([res[i]["out"].reshape(H, W_, C) for i in range(NCORES)])
    return out.astype(np.float32)


if __name__ == "__main__":
    rng = np.random.default_rng(0)
    x = rng.standard_normal((B, H, W_, C)).astype(np.float32)
    W = rng.standard_normal((MODES, MODES, C, C)).astype(np.float32) * 0.125
    out = kernel(x, W)
    print(out.shape, out.dtype)

